# revision 16
# baseline (speedup 1.0000x reference)
"""MoE routing kernel for Trainium2 (Bass/Tile), 8-core data-parallel.

Problem (hardcoded): B=131072, D=128, H=256, E=8.
  gates   = softmax(cluster_probs)                       [B, E]
  h       = relu(x @ W1[e] + b1[e])                      per expert
  y_e     = sigmoid(h @ W2[e] + b2[e])                   [E, B, 1]
  pred    = sum_e gates[:, e] * y_e                      [B, 1]
  experts_used          = sum_e (gates > 0.01)           [B]
  expert_influence      = sum_b gates                    [E]
  expert_activation_count = sum_b (gates > 0.01)         [E]

Sharding: batch split across 8 cores (16384 tokens each); expert weights
replicated. Host transposes x -> xT per shard and permutes cluster_probs so
every device DMA runs at line rate; outputs come back in staged layouts and
are un-permuted on the host. The tiny [E] reductions are summed on the host.

Device compute layout (per core): 32 token-tiles of T=512.
  L1: for each of 16 (expert, h-half) chunks: psum_h[128h, 512t] =
      W1chunk[128d,128h].T @ xT[128d,512t]   (float32r, 1 cyc/row)
  relu(+b1) -> h_sb (ACT/DVE alternating)
  L2: psum_y[8e, 512t] += W2blk_c[128h, 8e].T @ h_sb  (accumulated, 16 chunks)
  sigmoid via 1/(1+exp(-z)): ACT stays on the Exp table set the whole kernel.
  gates: softmax without max-subtraction (|logits| < 6, fp32-safe; ACT exp is
  <=2 ULP so the 0.01-threshold outcomes match the reference bit-for-bit on
  this data distribution).
"""

import sys

sys.path.insert(0, "/opt/trn_rl_repo")

import numpy as np

from concourse import bacc, bass, masks, mybir
from concourse.tile import TileContext

F32 = mybir.dt.float32
F32R = mybir.dt.float32r

B, D, H, E = 131072, 128, 256, 8
N_CORES = 8
B_SH = B // N_CORES          # 16384 tokens per core
T = 512                      # tokens per tile
N_TILES = B_SH // T          # 32
N_SUB = T // 128             # 4 subtiles of 128 tokens
N_CHUNK = 16                 # (expert, h-half) chunks of 128 h-positions
GATE_THRESH = 0.01

_PROGRAM_CACHE = {}


def build_program(n_tiles=N_TILES):
    """Build the single-core SPMD Bass program (same NEFF on all 8 cores)."""
    b_sh = n_tiles * T
    # Bacc (not raw Bass): its compile() legalizes sync waits to the HW cap
    # (1 wait/instruction, via EventSemaphore splits), allocates registers,
    # and inserts ACT table loads.
    nc = bacc.Bacc()

    # ---- DRAM I/O (per-core shard shapes) ----
    xT_d = nc.dram_tensor("xT", [D, b_sh], F32, kind="ExternalInput")
    cp_d = nc.dram_tensor("cp", [128, n_tiles * 32], F32, kind="ExternalInput")
    w1_d = nc.dram_tensor("w1", [D, E * H], F32, kind="ExternalInput")
    # w2block | b1 | -b2 packed into one tensor: keeps total DMA count at 8
    # (one per DMAHW lane; lane reuse adds a second sync-wait this walrus
    # build rejects). fp32r slices are bitcast views of the f32 storage.
    wcb_d = nc.dram_tensor("wcb", [128, N_CHUNK * E + N_CHUNK + 1], F32, kind="ExternalInput")

    gates_o = nc.dram_tensor("gates_o", [128, n_tiles * 32], F32, kind="ExternalOutput")
    pred_o = nc.dram_tensor("pred_o", [N_SUB, n_tiles * 128], F32, kind="ExternalOutput")
    eu_o = nc.dram_tensor("eu_o", [N_SUB, n_tiles * 128], F32, kind="ExternalOutput")
    infl_o = nc.dram_tensor("infl_o", [E, 1], F32, kind="ExternalOutput")
    cnt_o = nc.dram_tensor("cnt_o", [E, 1], F32, kind="ExternalOutput")

    with TileContext(nc) as tc:
        with (
            tc.tile_pool(name="const", bufs=1) as constp,
            tc.tile_pool(name="xq", bufs=1) as xqp,
            tc.tile_pool(name="stage", bufs=1) as stagep,
            tc.tile_pool(name="h", bufs=4) as hp,
            tc.tile_pool(name="y", bufs=2) as yp,
            tc.tile_pool(name="gsc", bufs=3) as gscp,
            tc.tile_pool(name="ph", bufs=4, space="PSUM") as php,
            tc.tile_pool(name="py", bufs=2, space="PSUM") as pyp,
            tc.tile_pool(name="ps", bufs=2, space="PSUM") as psp,
        ):
            # ---- constants / weights resident in SBUF ----
            # fp32r matmul operands must come from a rounding producer;
            # SWDGE cast-DMAs (f32 dram -> f32r sbuf) round inline.
            w1_sb = constp.tile([D, E * H], F32R)
            nc.gpsimd.dma_start(out=w1_sb[:], in_=w1_d[:])
            wcb_sb = constp.tile([128, N_CHUNK * E + N_CHUNK + 1], F32R)
            nc.gpsimd.dma_start(out=wcb_sb[:], in_=wcb_d[:])
            w2b_sb = wcb_sb[:, 0 : N_CHUNK * E]
            b1_sb = wcb_sb[:, N_CHUNK * E : N_CHUNK * E + N_CHUNK].bitcast(F32)
            b2n_sb = wcb_sb[0:E, N_CHUNK * E + N_CHUNK : N_CHUNK * E + N_CHUNK + 1].bitcast(F32)
            ident = constp.tile([128, 128], F32)
            masks.make_identity(nc, ident[:])

            # whole-shard cluster-prob staging (pre-permuted on host)
            cp_all = stagep.tile([128, n_tiles * 32], F32)
            nc.sync.dma_start(out=cp_all[:], in_=cp_d[:])

            # whole-shard xT slab (single DMA: concurrent slab DMAs would
            # round-robin the same SDMA engines anyway, arriving together)
            xq0 = xqp.tile([D, n_tiles * T], F32R)
            nc.gpsimd.dma_start(out=xq0[:], in_=xT_d[:])

            # --- PE semaphore funnel -------------------------------------
            # This walrus build allows only ONE sync-wait on a (self-loading)
            # Matmult. Make the PE observe each prologue semaphore via tiny
            # ops that each wait on exactly one new semaphore, so every real
            # matmul later needs at most one wait (its direct producer).
            # All funnel ops share one scratch tile: PE program order, no
            # slot waits. Transposes later depend only on DVE-produced data
            # and DVE-released slots, which coalesce into one DVE wait.
            scr_ps = psp.tile([128, 512], F32, tag="tp")
            nc.tensor.transpose(scr_ps[:, :128], ident[:, :128], ident[:])
            nc.tensor.matmul(
                scr_ps[:], w1_sb[:, :128], w1_sb[:, :512], start=True, stop=True
            )
            nc.tensor.matmul(
                scr_ps[:], w2b_sb[:, :128], w1_sb[:, :512], start=True, stop=True
            )
            nc.tensor.matmul(
                scr_ps[:], w1_sb[:, :128], xq0[:, :512], start=True, stop=True
            )

            # output staging
            gates_all = stagep.tile([128, n_tiles * 32], F32)
            pred_st = stagep.tile([N_SUB, n_tiles * 128], F32)
            eu_st = stagep.tile([N_SUB, n_tiles * 128], F32)
            acc_infl = stagep.tile([128, E], F32)
            acc_cnt = stagep.tile([128, E], F32)
            nc.vector.memset(acc_infl[:], 0.0)
            nc.vector.memset(acc_cnt[:], 0.0)

            for t in range(n_tiles):
                xT_tile = xq0[:, t * T : (t + 1) * T]

                # ======== gates path (softmax over E=8, no max-subtraction) ====
                cp_t = cp_all[:, t * 32 : (t + 1) * 32]
                g_t = gates_all[:, t * 32 : (t + 1) * 32]
                # exp on ACT (<=2 ULP) into a scratch tile so gates_all has a
                # single writer engine (DVE) -> its output DMA needs one wait
                exps = gscp.tile([128, 32], F32, tag="exps")
                nc.scalar.activation(exps[:], cp_t, mybir.ActivationFunctionType.Exp)
                ssum = gscp.tile([128, N_SUB], F32, tag="ssum")
                nc.vector.tensor_reduce(
                    ssum[:],
                    exps[:].rearrange("p (s e) -> p s e", s=N_SUB),
                    mybir.AxisListType.X,
                    mybir.AluOpType.add,
                )
                rinv = gscp.tile([128, N_SUB], F32, tag="rinv")
                nc.vector.reciprocal(rinv[:], ssum[:])
                for s in range(N_SUB):
                    nc.vector.tensor_scalar_mul(
                        g_t[:, s * 8 : (s + 1) * 8],
                        exps[:, s * 8 : (s + 1) * 8],
                        rinv[:, s : s + 1],
                    )
                # active mask, experts_used, accumulators
                act_t = gscp.tile([128, 32], F32, tag="act")
                nc.vector.tensor_scalar(
                    act_t[:], g_t, GATE_THRESH, None, mybir.AluOpType.is_gt
                )
                eu_t = gscp.tile([128, N_SUB], F32, tag="eu")
                nc.vector.tensor_reduce(
                    eu_t[:],
                    act_t[:].rearrange("p (s e) -> p s e", s=N_SUB),
                    mybir.AxisListType.X,
                    mybir.AluOpType.add,
                )
                tmp8 = gscp.tile([128, E], F32, tag="tmp8")
                nc.vector.tensor_reduce(
                    tmp8[:],
                    g_t.rearrange("p (s e) -> p e s", s=N_SUB),
                    mybir.AxisListType.X,
                    mybir.AluOpType.add,
                )
                nc.vector.tensor_tensor(
                    acc_infl[:], acc_infl[:], tmp8[:], mybir.AluOpType.add
                )
                tmp8b = gscp.tile([128, E], F32, tag="tmp8b")
                nc.vector.tensor_reduce(
                    tmp8b[:],
                    act_t[:].rearrange("p (s e) -> p e s", s=N_SUB),
                    mybir.AxisListType.X,
                    mybir.AluOpType.add,
                )
                nc.vector.tensor_tensor(
                    acc_cnt[:], acc_cnt[:], tmp8b[:], mybir.AluOpType.add
                )
                # experts_used -> transposed staging [4, 128] per tile
                eu_ps = psp.tile([N_SUB, 128], F32, tag="tp")
                nc.tensor.transpose(eu_ps[:], eu_t[:], ident[:])
                nc.vector.tensor_copy(
                    eu_st[:, t * 128 : (t + 1) * 128], eu_ps[:]
                )

                # ======== expert MLP =========================================
                py_t = pyp.tile([E, T], F32, tag="py")
                for c in range(N_CHUNK):
                    ph_t = php.tile([128, T], F32, tag="ph")
                    nc.tensor.matmul(
                        ph_t[:],
                        w1_sb[:, c * 128 : (c + 1) * 128],
                        xT_tile,
                        start=True,
                        stop=True,
                    )
                    h_t = hp.tile([128, T], F32R, tag="h")
                    if c % 2 == 0:
                        nc.scalar.activation(
                            h_t[:],
                            ph_t[:],
                            mybir.ActivationFunctionType.Relu,
                            bias=b1_sb[:, c : c + 1],
                        )
                    else:
                        nc.vector.tensor_scalar(
                            h_t[:],
                            ph_t[:],
                            b1_sb[:, c : c + 1],
                            0.0,
                            mybir.AluOpType.add,
                            mybir.AluOpType.max,
                        )
                    nc.tensor.matmul(
                        py_t[:],
                        w2b_sb[:, c * E : (c + 1) * E],
                        h_t[:],
                        start=(c == 0),
                        stop=(c == N_CHUNK - 1),
                        skip_group_check=True,
                    )

                # sigmoid(z + b2) = 1 / (1 + exp(-z - b2)); ACT stays on Exp set
                ynexp = yp.tile([E, T], F32, tag="ynexp")
                nc.scalar.activation(
                    ynexp[:],
                    py_t[:],
                    mybir.ActivationFunctionType.Exp,
                    bias=b2n_sb,
                    scale=-1.0,
                )
                ysig = yp.tile([E, T], F32, tag="ysig")
                nc.vector.tensor_scalar_add(ysig[:], ynexp[:], 1.0)
                nc.vector.reciprocal(ysig[:], ysig[:])

                # ======== combine: pred = sum_e gates * y ====================
                pred_t = gscp.tile([128, N_SUB], F32, tag="pred")
                for s in range(N_SUB):
                    yT_ps = psp.tile([128, E], F32, tag="tp")
                    nc.tensor.transpose(
                        yT_ps[:], ysig[:, s * 128 : (s + 1) * 128], ident[:E, :E]
                    )
                    # tensor_tensor_reduce crashes this runtime; use
                    # mult + reduce instead
                    scr = gscp.tile([128, E], F32, tag="scr")
                    nc.vector.tensor_tensor(
                        scr[:], yT_ps[:], g_t[:, s * 8 : (s + 1) * 8],
                        mybir.AluOpType.mult,
                    )
                    nc.vector.tensor_reduce(
                        pred_t[:, s : s + 1], scr[:],
                        mybir.AxisListType.X, mybir.AluOpType.add,
                    )
                pred_ps = psp.tile([N_SUB, 128], F32, tag="tp")
                nc.tensor.transpose(pred_ps[:], pred_t[:], ident[:])
                nc.vector.tensor_copy(
                    pred_st[:, t * 128 : (t + 1) * 128], pred_ps[:]
                )

            # ---- epilogue: [E] partials + output DMAs ----
            ai_ps = psp.tile([E, 128], F32, tag="tp")
            nc.tensor.transpose(ai_ps[:], acc_infl[:], ident[:])
            infl_sb = gscp.tile([E, 1], F32, tag="infl")
            nc.vector.tensor_reduce(
                infl_sb[:], ai_ps[:], mybir.AxisListType.X, mybir.AluOpType.add
            )
            nc.sync.dma_start(out=infl_o[:], in_=infl_sb[:])

            ac_ps = psp.tile([E, 128], F32, tag="tp")
            nc.tensor.transpose(ac_ps[:], acc_cnt[:], ident[:])
            cnt_sb = gscp.tile([E, 1], F32, tag="cnt")
            nc.vector.tensor_reduce(
                cnt_sb[:], ac_ps[:], mybir.AxisListType.X, mybir.AluOpType.add
            )
            nc.sync.dma_start(out=cnt_o[:], in_=cnt_sb[:])

            nc.sync.dma_start(out=gates_o[:], in_=gates_all[:])
            nc.sync.dma_start(out=pred_o[:], in_=pred_st[:])
            nc.sync.dma_start(out=eu_o[:], in_=eu_st[:])

    nc.compile()
    return nc


def _prep_shared(W1, b1, W2, b2):
    """Host-side weight layouts shared by all cores."""
    w1h = np.ascontiguousarray(
        W1.transpose(1, 0, 2).reshape(D, E * H), dtype=np.float32
    )
    b1h = np.ascontiguousarray(
        b1.reshape(E, 2, 128).transpose(2, 0, 1).reshape(128, N_CHUNK),
        dtype=np.float32,
    )
    w2b = np.zeros((128, N_CHUNK * E), dtype=np.float32)
    w2f = W2.reshape(E, H)  # [E, 256]
    for c in range(N_CHUNK):
        e, half = c // 2, c % 2
        w2b[:, c * E + e] = w2f[e, half * 128 : half * 128 + 128]
    # pack [w2block | b1 | -b2] into one [128, 145] tensor (single DMA)
    wcb = np.zeros((128, N_CHUNK * E + N_CHUNK + 1), dtype=np.float32)
    wcb[:, : N_CHUNK * E] = w2b
    wcb[:, N_CHUNK * E : N_CHUNK * E + N_CHUNK] = b1h
    wcb[:E, N_CHUNK * E + N_CHUNK] = -b2.reshape(E)
    return w1h, wcb


def _prep_core_inputs(x_sh, cp_sh, shared, n_tiles=N_TILES):
    w1h, wcb = shared
    xT = np.ascontiguousarray(x_sh.T, dtype=np.float32)  # [128, b_sh]
    # cp permuted: out[p, t*32 + s*8 + e] = cp[t*512 + s*128 + p, e]
    cph = np.ascontiguousarray(
        cp_sh.reshape(n_tiles, N_SUB, 128, E)
        .transpose(2, 0, 1, 3)
        .reshape(128, n_tiles * 32),
        dtype=np.float32,
    )
    return {"xT": xT, "cp": cph, "w1": w1h, "wcb": wcb}


def _unpack_core_outputs(res, n_tiles=N_TILES):
    b_sh = n_tiles * T
    gates = (
        res["gates_o"]
        .reshape(128, n_tiles, N_SUB, E)
        .transpose(1, 2, 0, 3)
        .reshape(b_sh, E)
    )
    pred = (
        res["pred_o"].reshape(N_SUB, n_tiles, 128).transpose(1, 0, 2).reshape(b_sh, 1)
    )
    eu = res["eu_o"].reshape(N_SUB, n_tiles, 128).transpose(1, 0, 2).reshape(b_sh)
    return gates, pred, eu, res["infl_o"].reshape(E), res["cnt_o"].reshape(E)


LAST_RESULTS = None  # test.py reads exec_time_ns / trace info from here


def kernel(x, cluster_probs, W1, b1, W2, b2, _trace=False, _trace_kwargs=None):
    global LAST_RESULTS
    from concourse.bass_utils import run_bass_kernel_spmd

    x = np.asarray(x, dtype=np.float32)
    cluster_probs = np.asarray(cluster_probs, dtype=np.float32)
    W1 = np.asarray(W1, dtype=np.float32)
    b1 = np.asarray(b1, dtype=np.float32)
    W2 = np.asarray(W2, dtype=np.float32)
    b2 = np.asarray(b2, dtype=np.float32)

    if "prog" not in _PROGRAM_CACHE:
        _PROGRAM_CACHE["prog"] = build_program()
    nc = _PROGRAM_CACHE["prog"]

    shared = _prep_shared(W1, b1, W2, b2)
    in_maps = []
    for c in range(N_CORES):
        sl = slice(c * B_SH, (c + 1) * B_SH)
        in_maps.append(_prep_core_inputs(x[sl], cluster_probs[sl], shared))

    kw = dict(_trace_kwargs or {})
    res = run_bass_kernel_spmd(
        nc, in_maps, core_ids=list(range(N_CORES)), trace=_trace, **kw
    )
    LAST_RESULTS = res

    gates = np.empty((B, E), dtype=np.float32)
    pred = np.empty((B, 1), dtype=np.float32)
    eu = np.empty((B,), dtype=np.float32)
    infl = np.zeros((E,), dtype=np.float32)
    cnt = np.zeros((E,), dtype=np.float32)
    for c in range(N_CORES):
        g, p, u, fi, fc = _unpack_core_outputs(res.results[c])
        sl = slice(c * B_SH, (c + 1) * B_SH)
        gates[sl] = g
        pred[sl] = p
        eu[sl] = u
        infl += fi
        cnt += fc
    return (pred, gates, eu, infl, cnt)


# revision 17
# speedup vs baseline: 4.8442x; 4.8442x over previous
"""MoE routing kernel for Trainium2 (Bass/Tile), 8-core data-parallel.

Problem (hardcoded): B=131072, D=128, H=256, E=8.
  gates   = softmax(cluster_probs)                       [B, E]
  h       = relu(x @ W1[e] + b1[e])                      per expert
  y_e     = sigmoid(h @ W2[e] + b2[e])                   [E, B, 1]
  pred    = sum_e gates[:, e] * y_e                      [B, 1]
  experts_used          = sum_e (gates > 0.01)           [B]
  expert_influence      = sum_b gates                    [E]
  expert_activation_count = sum_b (gates > 0.01)         [E]

Sharding: batch split across 8 cores (16384 tokens each); expert weights
replicated. Host transposes x -> xT per shard and permutes cluster_probs so
every device DMA runs at line rate; outputs come back in staged layouts and
are un-permuted on the host. The tiny [E] reductions are summed on the host.

Device compute layout (per core): 32 token-tiles of T=512.
  L1: for each of 16 (expert, h-half) chunks: psum_h[128h, 512t] =
      W1chunk[128d,128h].T @ xT[128d,512t]   (float32r, 1 cyc/row)
  relu(+b1) -> h_sb (ACT/DVE alternating)
  L2: psum_y[8e, 512t] += W2blk_c[128h, 8e].T @ h_sb  (accumulated, 16 chunks)
  sigmoid via 1/(1+exp(-z)): ACT stays on the Exp table set the whole kernel.
  gates: softmax without max-subtraction (|logits| < 6, fp32-safe; ACT exp is
  <=2 ULP so the 0.01-threshold outcomes match the reference bit-for-bit on
  this data distribution).
"""

import sys

sys.path.insert(0, "/opt/trn_rl_repo")

import numpy as np

from concourse import bacc, bass, masks, mybir
from concourse.tile import TileContext

F32 = mybir.dt.float32
F32R = mybir.dt.float32r

B, D, H, E = 131072, 128, 256, 8
N_CORES = 8
B_SH = B // N_CORES          # 16384 tokens per core
T = 512                      # tokens per tile
N_TILES = B_SH // T          # 32
N_SUB = T // 128             # 4 subtiles of 128 tokens
N_CHUNK = 16                 # (expert, h-half) chunks of 128 h-positions
GATE_THRESH = 0.01

_PROGRAM_CACHE = {}


def build_program(n_tiles=N_TILES, n_repeat=1):
    """Build the single-core SPMD Bass program (same NEFF on all 8 cores).

    n_repeat > 1 re-runs the whole tile loop inside one NEFF (outputs just
    overwritten) — used by the span benchmark to amortize away the ~2ms
    per-dispatch overhead of the axon tunnel, which otherwise hides the
    device-side kernel span entirely.
    """
    b_sh = n_tiles * T
    # Bacc (not raw Bass): its compile() legalizes sync waits to the HW cap
    # (1 wait/instruction, via EventSemaphore splits), allocates registers,
    # and inserts ACT table loads.
    nc = bacc.Bacc()

    # ---- DRAM I/O (per-core shard shapes) ----
    xT_d = nc.dram_tensor("xT", [D, b_sh], F32, kind="ExternalInput")
    cp_d = nc.dram_tensor("cp", [128, n_tiles * 32], F32, kind="ExternalInput")
    w1_d = nc.dram_tensor("w1", [D, E * H], F32, kind="ExternalInput")
    # w2block | b1 | -b2 packed into one tensor: keeps total DMA count at 8
    # (one per DMAHW lane; lane reuse adds a second sync-wait this walrus
    # build rejects). fp32r slices are bitcast views of the f32 storage.
    wcb_d = nc.dram_tensor("wcb", [128, N_CHUNK * E + N_CHUNK + 1], F32, kind="ExternalInput")

    gates_o = nc.dram_tensor("gates_o", [128, n_tiles * 32], F32, kind="ExternalOutput")
    pred_o = nc.dram_tensor("pred_o", [N_SUB, n_tiles * 128], F32, kind="ExternalOutput")
    eu_o = nc.dram_tensor("eu_o", [N_SUB, n_tiles * 128], F32, kind="ExternalOutput")
    infl_o = nc.dram_tensor("infl_o", [E, 1], F32, kind="ExternalOutput")
    cnt_o = nc.dram_tensor("cnt_o", [E, 1], F32, kind="ExternalOutput")

    with TileContext(nc) as tc:
        with (
            tc.tile_pool(name="const", bufs=1) as constp,
            tc.tile_pool(name="xq", bufs=1) as xqp,
            tc.tile_pool(name="stage", bufs=1) as stagep,
            tc.tile_pool(name="h", bufs=4) as hp,
            tc.tile_pool(name="y", bufs=2) as yp,
            tc.tile_pool(name="gsc", bufs=3) as gscp,
            tc.tile_pool(name="ph", bufs=4, space="PSUM") as php,
            tc.tile_pool(name="py", bufs=2, space="PSUM") as pyp,
            tc.tile_pool(name="ps", bufs=2, space="PSUM") as psp,
        ):
            # ---- constants / weights resident in SBUF ----
            # fp32r matmul operands must come from a rounding producer;
            # SWDGE cast-DMAs (f32 dram -> f32r sbuf) round inline.
            w1_sb = constp.tile([D, E * H], F32R)
            nc.gpsimd.dma_start(out=w1_sb[:], in_=w1_d[:])
            wcb_sb = constp.tile([128, N_CHUNK * E + N_CHUNK + 1], F32R)
            nc.gpsimd.dma_start(out=wcb_sb[:], in_=wcb_d[:])
            w2b_sb = wcb_sb[:, 0 : N_CHUNK * E]
            b1_sb = wcb_sb[:, N_CHUNK * E : N_CHUNK * E + N_CHUNK].bitcast(F32)
            b2n_sb = wcb_sb[0:E, N_CHUNK * E + N_CHUNK : N_CHUNK * E + N_CHUNK + 1].bitcast(F32)
            ident = constp.tile([128, 128], F32)
            masks.make_identity(nc, ident[:])

            # whole-shard cluster-prob staging (pre-permuted on host)
            cp_all = stagep.tile([128, n_tiles * 32], F32)
            nc.sync.dma_start(out=cp_all[:], in_=cp_d[:])

            # whole-shard xT slab (single DMA: concurrent slab DMAs would
            # round-robin the same SDMA engines anyway, arriving together)
            xq0 = xqp.tile([D, n_tiles * T], F32R)
            nc.gpsimd.dma_start(out=xq0[:], in_=xT_d[:])

            # --- PE semaphore funnel -------------------------------------
            # This walrus build allows only ONE sync-wait on a (self-loading)
            # Matmult. Make the PE observe each prologue semaphore via tiny
            # ops that each wait on exactly one new semaphore, so every real
            # matmul later needs at most one wait (its direct producer).
            # All funnel ops share one scratch tile: PE program order, no
            # slot waits. Transposes later depend only on DVE-produced data
            # and DVE-released slots, which coalesce into one DVE wait.
            scr_ps = psp.tile([128, 512], F32, tag="tp")
            nc.tensor.transpose(scr_ps[:, :128], ident[:, :128], ident[:])
            nc.tensor.matmul(
                scr_ps[:], w1_sb[:, :128], w1_sb[:, :512], start=True, stop=True
            )
            nc.tensor.matmul(
                scr_ps[:], w2b_sb[:, :128], w1_sb[:, :512], start=True, stop=True
            )
            nc.tensor.matmul(
                scr_ps[:], w1_sb[:, :128], xq0[:, :512], start=True, stop=True
            )

            # output staging
            gates_all = stagep.tile([128, n_tiles * 32], F32)
            pred_st = stagep.tile([N_SUB, n_tiles * 128], F32)
            eu_st = stagep.tile([N_SUB, n_tiles * 128], F32)
            acc_infl = stagep.tile([128, E], F32)
            acc_cnt = stagep.tile([128, E], F32)
            nc.vector.memset(acc_infl[:], 0.0)
            nc.vector.memset(acc_cnt[:], 0.0)

            for rep_t in range(n_repeat * n_tiles):
                t = rep_t % n_tiles
                xT_tile = xq0[:, t * T : (t + 1) * T]

                # ======== gates path (softmax over E=8, no max-subtraction) ====
                cp_t = cp_all[:, t * 32 : (t + 1) * 32]
                g_t = gates_all[:, t * 32 : (t + 1) * 32]
                # exp on ACT (<=2 ULP) into a scratch tile so gates_all has a
                # single writer engine (DVE) -> its output DMA needs one wait
                exps = gscp.tile([128, 32], F32, tag="exps")
                nc.scalar.activation(exps[:], cp_t, mybir.ActivationFunctionType.Exp)
                ssum = gscp.tile([128, N_SUB], F32, tag="ssum")
                nc.vector.tensor_reduce(
                    ssum[:],
                    exps[:].rearrange("p (s e) -> p s e", s=N_SUB),
                    mybir.AxisListType.X,
                    mybir.AluOpType.add,
                )
                rinv = gscp.tile([128, N_SUB], F32, tag="rinv")
                nc.vector.reciprocal(rinv[:], ssum[:])
                for s in range(N_SUB):
                    nc.vector.tensor_scalar_mul(
                        g_t[:, s * 8 : (s + 1) * 8],
                        exps[:, s * 8 : (s + 1) * 8],
                        rinv[:, s : s + 1],
                    )
                # active mask, experts_used, accumulators
                act_t = gscp.tile([128, 32], F32, tag="act")
                nc.vector.tensor_scalar(
                    act_t[:], g_t, GATE_THRESH, None, mybir.AluOpType.is_gt
                )
                eu_t = gscp.tile([128, N_SUB], F32, tag="eu")
                nc.vector.tensor_reduce(
                    eu_t[:],
                    act_t[:].rearrange("p (s e) -> p s e", s=N_SUB),
                    mybir.AxisListType.X,
                    mybir.AluOpType.add,
                )
                tmp8 = gscp.tile([128, E], F32, tag="tmp8")
                nc.vector.tensor_reduce(
                    tmp8[:],
                    g_t.rearrange("p (s e) -> p e s", s=N_SUB),
                    mybir.AxisListType.X,
                    mybir.AluOpType.add,
                )
                nc.vector.tensor_tensor(
                    acc_infl[:], acc_infl[:], tmp8[:], mybir.AluOpType.add
                )
                tmp8b = gscp.tile([128, E], F32, tag="tmp8b")
                nc.vector.tensor_reduce(
                    tmp8b[:],
                    act_t[:].rearrange("p (s e) -> p e s", s=N_SUB),
                    mybir.AxisListType.X,
                    mybir.AluOpType.add,
                )
                nc.vector.tensor_tensor(
                    acc_cnt[:], acc_cnt[:], tmp8b[:], mybir.AluOpType.add
                )
                # experts_used -> transposed staging [4, 128] per tile
                eu_ps = psp.tile([N_SUB, 128], F32, tag="tp")
                nc.tensor.transpose(eu_ps[:], eu_t[:], ident[:])
                nc.vector.tensor_copy(
                    eu_st[:, t * 128 : (t + 1) * 128], eu_ps[:]
                )

                # ======== expert MLP =========================================
                py_t = pyp.tile([E, T], F32, tag="py")
                for c in range(N_CHUNK):
                    ph_t = php.tile([128, T], F32, tag="ph")
                    nc.tensor.matmul(
                        ph_t[:],
                        w1_sb[:, c * 128 : (c + 1) * 128],
                        xT_tile,
                        start=True,
                        stop=True,
                    )
                    h_t = hp.tile([128, T], F32R, tag="h")
                    if c % 2 == 0:
                        nc.scalar.activation(
                            h_t[:],
                            ph_t[:],
                            mybir.ActivationFunctionType.Relu,
                            bias=b1_sb[:, c : c + 1],
                        )
                    else:
                        nc.vector.tensor_scalar(
                            h_t[:],
                            ph_t[:],
                            b1_sb[:, c : c + 1],
                            0.0,
                            mybir.AluOpType.add,
                            mybir.AluOpType.max,
                        )
                    nc.tensor.matmul(
                        py_t[:],
                        w2b_sb[:, c * E : (c + 1) * E],
                        h_t[:],
                        start=(c == 0),
                        stop=(c == N_CHUNK - 1),
                        skip_group_check=True,
                    )

                # sigmoid(z + b2) = 1 / (1 + exp(-z - b2)); ACT stays on Exp set
                ynexp = yp.tile([E, T], F32, tag="ynexp")
                nc.scalar.activation(
                    ynexp[:],
                    py_t[:],
                    mybir.ActivationFunctionType.Exp,
                    bias=b2n_sb,
                    scale=-1.0,
                )
                ysig = yp.tile([E, T], F32, tag="ysig")
                nc.vector.tensor_scalar_add(ysig[:], ynexp[:], 1.0)
                nc.vector.reciprocal(ysig[:], ysig[:])

                # ======== combine: pred = sum_e gates * y ====================
                pred_t = gscp.tile([128, N_SUB], F32, tag="pred")
                for s in range(N_SUB):
                    yT_ps = psp.tile([128, E], F32, tag="tp")
                    nc.tensor.transpose(
                        yT_ps[:], ysig[:, s * 128 : (s + 1) * 128], ident[:E, :E]
                    )
                    # tensor_tensor_reduce crashes this runtime; use
                    # mult + reduce instead
                    scr = gscp.tile([128, E], F32, tag="scr")
                    nc.vector.tensor_tensor(
                        scr[:], yT_ps[:], g_t[:, s * 8 : (s + 1) * 8],
                        mybir.AluOpType.mult,
                    )
                    nc.vector.tensor_reduce(
                        pred_t[:, s : s + 1], scr[:],
                        mybir.AxisListType.X, mybir.AluOpType.add,
                    )
                pred_ps = psp.tile([N_SUB, 128], F32, tag="tp")
                nc.tensor.transpose(pred_ps[:], pred_t[:], ident[:])
                nc.vector.tensor_copy(
                    pred_st[:, t * 128 : (t + 1) * 128], pred_ps[:]
                )

            # ---- epilogue: [E] partials + output DMAs ----
            ai_ps = psp.tile([E, 128], F32, tag="tp")
            nc.tensor.transpose(ai_ps[:], acc_infl[:], ident[:])
            infl_sb = gscp.tile([E, 1], F32, tag="infl")
            nc.vector.tensor_reduce(
                infl_sb[:], ai_ps[:], mybir.AxisListType.X, mybir.AluOpType.add
            )
            nc.sync.dma_start(out=infl_o[:], in_=infl_sb[:])

            ac_ps = psp.tile([E, 128], F32, tag="tp")
            nc.tensor.transpose(ac_ps[:], acc_cnt[:], ident[:])
            cnt_sb = gscp.tile([E, 1], F32, tag="cnt")
            nc.vector.tensor_reduce(
                cnt_sb[:], ac_ps[:], mybir.AxisListType.X, mybir.AluOpType.add
            )
            nc.sync.dma_start(out=cnt_o[:], in_=cnt_sb[:])

            nc.sync.dma_start(out=gates_o[:], in_=gates_all[:])
            nc.sync.dma_start(out=pred_o[:], in_=pred_st[:])
            nc.sync.dma_start(out=eu_o[:], in_=eu_st[:])

    nc.compile()
    return nc


def _prep_shared(W1, b1, W2, b2):
    """Host-side weight layouts shared by all cores."""
    w1h = np.ascontiguousarray(
        W1.transpose(1, 0, 2).reshape(D, E * H), dtype=np.float32
    )
    b1h = np.ascontiguousarray(
        b1.reshape(E, 2, 128).transpose(2, 0, 1).reshape(128, N_CHUNK),
        dtype=np.float32,
    )
    w2b = np.zeros((128, N_CHUNK * E), dtype=np.float32)
    w2f = W2.reshape(E, H)  # [E, 256]
    for c in range(N_CHUNK):
        e, half = c // 2, c % 2
        w2b[:, c * E + e] = w2f[e, half * 128 : half * 128 + 128]
    # pack [w2block | b1 | -b2] into one [128, 145] tensor (single DMA)
    wcb = np.zeros((128, N_CHUNK * E + N_CHUNK + 1), dtype=np.float32)
    wcb[:, : N_CHUNK * E] = w2b
    wcb[:, N_CHUNK * E : N_CHUNK * E + N_CHUNK] = b1h
    wcb[:E, N_CHUNK * E + N_CHUNK] = -b2.reshape(E)
    return w1h, wcb


def _prep_core_inputs(x_sh, cp_sh, shared, n_tiles=N_TILES):
    w1h, wcb = shared
    xT = np.ascontiguousarray(x_sh.T, dtype=np.float32)  # [128, b_sh]
    # cp permuted: out[p, t*32 + s*8 + e] = cp[t*512 + s*128 + p, e]
    cph = np.ascontiguousarray(
        cp_sh.reshape(n_tiles, N_SUB, 128, E)
        .transpose(2, 0, 1, 3)
        .reshape(128, n_tiles * 32),
        dtype=np.float32,
    )
    return {"xT": xT, "cp": cph, "w1": w1h, "wcb": wcb}


def _unpack_core_outputs(res, n_tiles=N_TILES):
    b_sh = n_tiles * T
    gates = (
        res["gates_o"]
        .reshape(128, n_tiles, N_SUB, E)
        .transpose(1, 2, 0, 3)
        .reshape(b_sh, E)
    )
    pred = (
        res["pred_o"].reshape(N_SUB, n_tiles, 128).transpose(1, 0, 2).reshape(b_sh, 1)
    )
    eu = res["eu_o"].reshape(N_SUB, n_tiles, 128).transpose(1, 0, 2).reshape(b_sh)
    return gates, pred, eu, res["infl_o"].reshape(E), res["cnt_o"].reshape(E)


LAST_RESULTS = None  # test.py reads exec_time_ns / trace info from here


def kernel(x, cluster_probs, W1, b1, W2, b2, _trace=False, _trace_kwargs=None):
    global LAST_RESULTS
    from concourse.bass_utils import run_bass_kernel_spmd

    x = np.asarray(x, dtype=np.float32)
    cluster_probs = np.asarray(cluster_probs, dtype=np.float32)
    W1 = np.asarray(W1, dtype=np.float32)
    b1 = np.asarray(b1, dtype=np.float32)
    W2 = np.asarray(W2, dtype=np.float32)
    b2 = np.asarray(b2, dtype=np.float32)

    if "prog" not in _PROGRAM_CACHE:
        _PROGRAM_CACHE["prog"] = build_program()
    nc = _PROGRAM_CACHE["prog"]

    shared = _prep_shared(W1, b1, W2, b2)
    in_maps = []
    for c in range(N_CORES):
        sl = slice(c * B_SH, (c + 1) * B_SH)
        in_maps.append(_prep_core_inputs(x[sl], cluster_probs[sl], shared))

    kw = dict(_trace_kwargs or {})
    res = run_bass_kernel_spmd(
        nc, in_maps, core_ids=list(range(N_CORES)), trace=_trace, **kw
    )
    LAST_RESULTS = res

    gates = np.empty((B, E), dtype=np.float32)
    pred = np.empty((B, 1), dtype=np.float32)
    eu = np.empty((B,), dtype=np.float32)
    infl = np.zeros((E,), dtype=np.float32)
    cnt = np.zeros((E,), dtype=np.float32)
    for c in range(N_CORES):
        g, p, u, fi, fc = _unpack_core_outputs(res.results[c])
        sl = slice(c * B_SH, (c + 1) * B_SH)
        gates[sl] = g
        pred[sl] = p
        eu[sl] = u
        infl += fi
        cnt += fc
    return (pred, gates, eu, infl, cnt)
